# revision 64
# baseline (speedup 1.0000x reference)
"""Multi-head attention TRN2 kernel (8 NeuronCores).

Sharding: core (2b + h2) handles batch b (of 4) and head-half h2 (8 of 16
heads).  Each core projects its batch's Q/K/V through its 512-column slice
of Wq/Wk/Wv, runs causal flash-attention for its 8 heads, and computes a
partial output projection through its 512 rows of Wo^T.  The two partial
outputs per batch are summed on the host (the "all-reduce after W_o"),
plus the output bias.

All matmuls run in fp16 with fp32 PSUM accumulation.  Scores are computed
transposed (S^T[kj, qi] = kT.T @ qT) so the softmax sum comes for free from
a ones-column appended to V (padded to 128 columns so LDWEIGHTS gets fast
weight load), and the attention output lands f-major, which is exactly the
lhsT layout the Wo matmul needs.  Normalisation (divide by the softmax sum)
is a DVE fast-reciprocal (read straight from PSUM) + GPSIMD
partition-broadcast + DVE multiply.

Causal handling: per 512-token qi block the four diagonal 512x512 kj tiles
are trimmed to their live qi ranges (widths 512/384/256/128 for scores,
exp and PV; PSUM start zeroes the whole 2KB zero region so partial-width
accumulation is safe), and only the 128-wide boundary triangle of each
tile is masked.  Off-diagonal upper tiles are skipped entirely.

The schedule is a flat (head, chunk) software pipeline per qi block with
a deep pending-PV queue (lookahead 5, spanning head boundaries) so every
PV's es-wait resolves long before the PE reaches it; each head's
normalisation is emitted when its final PV pops.  Chunk-0 projections are
emitted K-first with host-side tensors laid out partition-major (every
DMA contiguous per partition, cheap descriptor generation) and startup
triggers split across the two HWDGE queues (SP + ACT).  Projection
pieces for chunks 1..3 and the Wo pieces for qi blocks 0..2 are spread
as PE filler across the attention blocks in proportion to each block's
PE-vs-ACT deficit (late blocks have more exp work per matmul); the k/q
projection biases are split across the ACT/DVE engines.  Partial outputs
stream back as fp16 and the two per-batch partials are summed on the
host.
"""

import os
import sys

sys.path.insert(0, "/opt/trn_rl_repo")

import numpy as np

import concourse.bass as bass
import concourse.mybir as mybir
import concourse.tile as tile
from concourse import bacc
from concourse.bass_utils import run_bass_kernel_spmd

F16 = mybir.dt.float16
F32 = mybir.dt.float32
P = 128

# Problem constants (full size).
D_MODEL = 1024
NUM_HEADS = 16
DK = D_MODEL // NUM_HEADS  # 64
BATCH = 4
SEQ = 2048
N_CORES = 8

LAST_EXEC_NS = None
LAST_RESULTS = None


def build_program(seq=SEQ, d_model=D_MODEL, num_heads=NUM_HEADS, mode="causal"):
    """Build the per-core Bass program.  Uniform across cores (SPMD).

    mode: "causal" (tril mask, block-skip + trimmed diagonal),
          "dense"  (no mask),
          "mask"   (arbitrary 0/1 mask, multiplicative, streamed from DRAM).
    """
    assert d_model % 256 == 0 and seq % P == 0
    HL = num_heads // 2              # local heads per core
    PAIRS = HL // 2                  # head-pairs (128 partitions each)
    FL = HL * DK                     # local features (columns of W slices)
    IN_T = d_model // P              # input-dim tiles
    FT = FL // P                     # local f tiles
    TT = seq // P                    # token tiles
    QBS = min(512, seq)              # qi block size
    QB = seq // QBS                  # qi blocks
    KJ = seq // P                    # key tiles
    KPB = QBS // P                   # key tiles per qi block (diag width)
    OFC = (d_model + 511) // 512     # output-feature chunks
    OFS = min(512, d_model)
    assert PAIRS >= 1 and FT >= 1 and QB >= 1

    nc = bacc.Bacc()
    # x tensors partition-major: [P, QB*IN_T*QBS], chunk ch contiguous.
    xtq = nc.declare_dram_parameter("xtq", [P, QB * IN_T * QBS], F16,
                                    isOutput=False)
    xtk = nc.declare_dram_parameter("xtk", [P, QB * IN_T * QBS], F16,
                                    isOutput=False)
    xtv = nc.declare_dram_parameter("xtv", [P, QB * IN_T * QBS], F16,
                                    isOutput=False)
    # weights partition-major: [P, IN_T*FL] / [P, FT*d_model].
    wqt = nc.declare_dram_parameter("wqt", [P, IN_T * FL], F16, isOutput=False)
    wkt = nc.declare_dram_parameter("wkt", [P, IN_T * FL], F16, isOutput=False)
    wvt = nc.declare_dram_parameter("wvt", [P, IN_T * FL], F16, isOutput=False)
    bkq = nc.declare_dram_parameter("bkq", [P, 2 * PAIRS], F32, isOutput=False)
    bvrow = nc.declare_dram_parameter("bvrow", [1, FL], F16, isOutput=False)
    wot = nc.declare_dram_parameter("wot", [P, FT * d_model], F16,
                                    isOutput=False)
    if mode == "mask":
        maskt = nc.declare_dram_parameter("maskt", [seq, seq], F16,
                                          isOutput=False)
    out = nc.declare_dram_parameter("out", [seq, d_model], F16, isOutput=True)

    AF = mybir.ActivationFunctionType

    def x_chunk_ap(xt, ch, it0=0, it1=None):
        it1 = IN_T if it1 is None else it1
        base = ch * IN_T * QBS
        return xt[:, base + it0 * QBS:base + it1 * QBS].rearrange(
            "p (it t) -> p it t", it=it1 - it0)

    with tile.TileContext(nc) as tc:
        with (
            tc.tile_pool(name="const", bufs=1) as cpool,
            tc.tile_pool(name="big", bufs=1) as big,
            tc.tile_pool(name="xs", bufs=2) as xs,
            tc.tile_pool(name="es", bufs=8) as esp,
            tc.tile_pool(name="ep", bufs=2) as epi,
            tc.tile_pool(name="osb", bufs=4) as osb,
        ):
            # ---- startup DMAs.  Both HWDGE queues (SP + ACT) carry the
            # K-path first (wk/xk split between them) so the first
            # accumulation groups are fed at full aggregate bandwidth,
            # then Q, then V.
            wk_sb = cpool.tile([P, IN_T, FL], F16, name="wk")
            xk0 = xs.tile([P, IN_T, QBS], F16, tag="xk", name="xk_0")
            H_IT = IN_T // 2
            nc.sync.dma_start(wk_sb[:, 0:H_IT, :],
                              wkt[:, 0:H_IT * FL].rearrange(
                                  "p (it f) -> p it f", it=H_IT))
            nc.scalar.dma_start(xk0[:, 0:H_IT, :], x_chunk_ap(xtk, 0, 0, H_IT))
            nc.sync.dma_start(wk_sb[:, H_IT:IN_T, :],
                              wkt[:, H_IT * FL:IN_T * FL].rearrange(
                                  "p (it f) -> p it f", it=IN_T - H_IT))
            nc.scalar.dma_start(xk0[:, H_IT:IN_T, :], x_chunk_ap(xtk, 0, H_IT))
            bkq_sb = cpool.tile([P, 2 * PAIRS], F32)
            nc.scalar.dma_start(bkq_sb[:], bkq[:, :])
            wq_sb = cpool.tile([P, IN_T, FL], F16, name="wq")
            nc.sync.dma_start(wq_sb[:],
                              wqt[:].rearrange("p (it f) -> p it f", it=IN_T))
            xq0 = xs.tile([P, IN_T, QBS], F16, tag="xq", name="xq_0")
            nc.scalar.dma_start(xq0[:], x_chunk_ap(xtq, 0))
            wv_sb = cpool.tile([P, IN_T, FL], F16, name="wv")
            nc.sync.dma_start(wv_sb[:],
                              wvt[:].rearrange("p (it f) -> p it f", it=IN_T))
            xv0 = xs.tile([P, IN_T, QBS], F16, tag="xv", name="xv_0")
            nc.scalar.dma_start(xv0[:], x_chunk_ap(xtv, 0))
            bv_sb = cpool.tile([1, FL], F16)
            nc.scalar.dma_start(bv_sb[:], bvrow[:, :])

            def bk_col(pair):
                return bkq_sb[:, pair:pair + 1]

            def bq_col(pair):
                return bkq_sb[:, PAIRS + pair:PAIRS + pair + 1]

            # wo is needed only ~150us in; its DMA is issued at the start
            # of the qb-0 attention block.
            wo_sb = cpool.tile([P, FT, d_model], F16)

            def emit_wo_dma():
                nc.sync.dma_start(
                    wo_sb[:],
                    wot[:].rearrange("p (ft o) -> p ft o", ft=FT))

            # ---- non-DMA constants ----
            ones1 = cpool.tile([1, P], F16)
            nc.gpsimd.memset(ones1[:], 1.0)
            # warm the ACT exp table early (one-time ~1.3us load)
            es_warm = esp.tile([1, 8], F16, tag="warm")
            nc.scalar.activation(es_warm[:], ones1[0:1, 0:8], AF.Exp, scale=1.0)
            bvb = cpool.tile([P, FL], F16)
            nc.gpsimd.partition_broadcast(bvb[:], bv_sb[0:1, :])
            # 4 diagonal 0/1 strips: strip j keeps (qi - kj_local - 128*j >= 0)
            mask4 = cpool.tile([P, KPB, QBS], F16)
            nc.gpsimd.memset(mask4[:], 1.0)
            for j in range(KPB):
                nc.gpsimd.affine_select(
                    out=mask4[:, j, :], in_=mask4[:, j, :],
                    compare_op=mybir.AluOpType.is_ge,
                    fill=0.0, base=-P * j,
                    pattern=[[1, QBS]], channel_multiplier=-1)

            # ---- persistent activations ----
            qT_sb = big.tile([P, PAIRS, seq], F16)   # [2-head f, pair, tok]
            kT_sb = big.tile([P, PAIRS, seq], F16)
            v_sb = big.tile([P, TT, HL, P], F16)  # [tok_in_tile, kj, h, d|1|pad]
            oT_sb = big.tile([P, FT, seq], F16)      # attention out, f-major

            nc.gpsimd.memset(v_sb[:], 0.0)
            nc.gpsimd.memset(v_sb[:, :, :, DK:DK + 1], 1.0)

            # One shared PSUM pool: tag "s" [128,2,512]x3 = 6 banks (scores,
            # projections, Wo) + tag "o" [128,512]x2 = 2 banks.
            pool_cm = tc.tile_pool(name="pmm", bufs=3, space="PSUM")
            pmm = pool_cm.__enter__()
            opool_cm = tc.tile_pool(name="po", bufs=2, space="PSUM")
            pop = opool_cm.__enter__()

            def emit_proj_dma(ch):
                    # xq/xv ride the ACT HWDGE queue: the SP queue
                    # otherwise carries ~3x the bytes and chunk V data
                    # lands several us after the PE needs it
                    xk_t = xs.tile([P, IN_T, QBS], F16, tag="xk",
                                   name=f"xk_{ch}")
                    nc.sync.dma_start(xk_t[:], x_chunk_ap(xtk, ch))
                    xq_t = xs.tile([P, IN_T, QBS], F16, tag="xq",
                                   name=f"xq_{ch}")
                    nc.scalar.dma_start(xq_t[:], x_chunk_ap(xtq, ch))
                    xv_t = xs.tile([P, IN_T, QBS], F16, tag="xv",
                                   name=f"xv_{ch}")
                    nc.scalar.dma_start(xv_t[:], x_chunk_ap(xtv, ch))
                    return xk_t, xq_t, xv_t

            def proj_pieces(ch, tiles=None):
                    tsl = slice(ch * QBS, (ch + 1) * QBS)
                    xk_t, xq_t, xv_t = tiles if tiles else emit_proj_dma(ch)
                    pieces = []
                    def qk_piece(pair):
                        def go():
                            _emit_qk_pair(ch, tsl, xk_t, xq_t, pair)
                        return go
                    def v_piece(tl):
                        def go():
                            _emit_v_tile(ch, tsl, xv_t, tl)
                        return go
                    for pair in range(PAIRS):
                        pieces.append(qk_piece(pair))
                    for tl in range(KPB):
                        pieces.append(v_piece(tl))
                    return pieces

            def emit_proj(ch, tiles=None):
                    for piece in proj_pieces(ch, tiles):
                        piece()

            def _emit_qk_pair(ch, tsl, xk_t, xq_t, pair):
                        fsl = slice(pair * P, (pair + 1) * P)
                        qk_ps = pmm.tile([P, 2, QBS], F32, tag="s",
                                         name=f"qk_{ch}_{pair}")
                        for it in range(IN_T):
                            nc.tensor.matmul(qk_ps[:, 0, :],
                                             wk_sb[:, it, fsl],
                                             xk_t[:, it, :],
                                             start=(it == 0), stop=(it == IN_T - 1))
                        for it in range(IN_T):
                            nc.tensor.matmul(qk_ps[:, 1, :],
                                             wq_sb[:, it, fsl],
                                             xq_t[:, it, :],
                                             start=(it == 0), stop=(it == IN_T - 1))
                        # bias + f32->f16 cast, split across ACT and DVE so
                        # neither the exp stream nor the norm chains (which
                        # gate o-PSUM rotation) queue behind both
                        nc.scalar.activation(kT_sb[:, pair, tsl],
                                             qk_ps[:, 0, :], AF.Identity,
                                             bias=bk_col(pair), scale=1.0)
                        nc.vector.tensor_scalar_add(qT_sb[:, pair, tsl],
                                                    qk_ps[:, 1, :],
                                                    bq_col(pair))

            def _emit_v_tile(ch, tsl, xv_t, tl):
                        v_ps = pmm.tile([P, 2, QBS], F32, tag="s",
                                        name=f"v_{ch}_{tl}")
                        for it in range(IN_T):
                            nc.tensor.matmul(
                                v_ps[:, 0, 0:FL],
                                xv_t[:, it, tl * P:(tl + 1) * P],
                                wv_sb[:, it, :],
                                start=(it == 0), stop=(it == IN_T - 1))
                        tt = ch * KPB + tl
                        nc.vector.tensor_tensor(
                            v_sb[:, tt, :, 0:DK],
                            v_ps[:, 0, 0:FL].rearrange("p (h d) -> p h d", h=HL),
                            bvb[:].rearrange("p (h d) -> p h d", h=HL),
                            mybir.AluOpType.add)

            def emit_chunk0(xk_t, xq_t, xv_t):
                    """Chunk-0 projections: K for all pairs, then Q pairs
                    0-1, so the first matmuls only wait on the xk/wk then
                    xq/wq DMAs.  Q pairs 2-3 and all V tiles are returned
                    as pieces scheduled inside the qb-0 attention stream
                    (the xv DMA is the last to land, and attention heads
                    0-3 only need q/k pairs 0-1)."""
                    tsl = slice(0, QBS)

                    def qk0_grp(wn, grp):
                        w_sb, xt, b_col, t_sb = (
                            (wk_sb, xk_t, bk_col, kT_sb),
                            (wq_sb, xq_t, bq_col, qT_sb))[wn]
                        ps = pmm.tile([P, 2, QBS], F32, tag="s",
                                      name=f"p0_{wn}_{grp}")
                        for i in range(2):
                            pair = grp * 2 + i
                            fsl = slice(pair * P, (pair + 1) * P)
                            for it in range(IN_T):
                                nc.tensor.matmul(
                                    ps[:, i, :],
                                    w_sb[:, it, fsl],
                                    xt[:, it, :],
                                    start=(it == 0), stop=(it == IN_T - 1))
                        for i in range(2):
                            pair = grp * 2 + i
                            if wn == 0:
                                nc.scalar.activation(
                                    t_sb[:, pair, tsl], ps[:, i, :],
                                    AF.Identity, bias=b_col(pair),
                                    scale=1.0)
                            else:
                                nc.vector.tensor_scalar_add(
                                    t_sb[:, pair, tsl], ps[:, i, :],
                                    b_col(pair))

                    for grp in range(PAIRS // 2):
                        qk0_grp(0, grp)
                    for grp in range(PAIRS // 2):
                        qk0_grp(1, grp)

                    def v_piece(tl):
                        def go():
                            _emit_v_tile(0, tsl, xv_t, tl)
                        return go
                    return [v_piece(tl) for tl in range(KPB)]

            # ---- attention for qi block qb, all local heads ----
            def attn_chunks(qb):
                """Chunk list [(kjs, kind)] for one qi block.  kind:
                'O' off-diagonal (full), 'DA' leading diag pair (widths
                512/384), 'DB' trailing diag pair (widths 256/128).  Diag
                tiles are trimmed to their live qi range; PSUM start=True
                zeroes the whole 2KB zero region so partial-width
                accumulation is safe, and stop bookkeeping is region-
                granular so a partial-width stop closes the group."""
                if mode != "causal":
                    chunks = []
                    for kj0 in range(0, KJ, 2):
                        kjs = (kj0, kj0 + 1) if kj0 + 1 < KJ else (kj0,)
                        chunks.append((kjs, 'O'))
                    return chunks
                d0 = qb * KPB
                DA = ((d0, d0 + 1), 'DA')
                DB = ((d0 + 2, d0 + 3), 'DB')
                if qb == 0:
                    return [DA, DB]
                # Diagonal (masked) chunks go second/third so their mask
                # multiplies never sit in the exposed exp->PV tail of the
                # head; the last off-diagonal chunk carries the stop.
                offs = [((kj0, kj0 + 1), 'O') for kj0 in range(0, d0, 2)]
                return [offs[0], DB, DA] + offs[1:]

            def emit_attn(qb, fillers=(), fill_from=0, cap=2,
                          fill_to=HL):
                fillers = list(fillers)
                qsl = slice(qb * QBS, (qb + 1) * QBS)
                chunks = attn_chunks(qb)
                nch = len(chunks)
                # odd chunk indices are filler slots (after that chunk's
                # scores, inside the exp->PV latency window)
                slots = [ci for ci in range(1, nch, 2)]
                # One flat (head, chunk) pipeline: the pending-PV queue
                # (lookahead `cap`) spans head boundaries, so the last PVs
                # of head h pop during head h+1's first scores instead of
                # stalling on their just-issued exps; each head's norm is
                # emitted when its final PV pops.
                pending = []  # (pv_fn, after_fn | None)

                def pump():
                    pv, after = pending.pop(0)
                    pv()
                    if after is not None:
                        after()

                def make_norm(o_ps, pair, po):
                    def go():
                        # normalise by the softmax sum (row DK), f-major:
                        # denominator row to a partition-0 tile for the
                        # custom-DVE reciprocal, GPSIMD-broadcast, then
                        # multiply straight from PSUM.
                        srow = epi.tile([1, QBS], F32, tag="srow")
                        nc.vector.tensor_copy(srow[:], o_ps[DK:DK + 1, :])
                        recip_row = epi.tile([1, QBS], F32, tag="recip_row")
                        nc.vector.reciprocal_approx_fast(recip_row[:],
                                                         srow[:])
                        recipb = epi.tile([DK, QBS], F32, tag="recipb")
                        nc.gpsimd.partition_broadcast(recipb[:],
                                                      recip_row[0:1, :])
                        nc.vector.tensor_mul(oT_sb[po:po + DK, pair, qsl],
                                             o_ps[0:DK, :], recipb[:])
                    return go

                for h in range(HL):
                    pair = h // 2
                    po = (h % 2) * DK
                    qT_h = qT_sb[po:po + DK, pair, qsl]
                    o_ps = pop.tile([P, QBS], F32, tag="o",
                                    name=f"o_{qb}_{h}")
                    # this head's share of the remaining fillers, spread
                    # over its slots (multiple pops per slot if needed)
                    if fillers and fill_from <= h < fill_to:
                        nf = -(-len(fillers) // (fill_to - h))
                    else:
                        nf = 0
                    slot_pops = {}
                    for j in range(nf):
                        ci_s = slots[j % len(slots)]
                        slot_pops[ci_s] = slot_pops.get(ci_s, 0) + 1
                    chunks_h = chunks
                    if (mode == "causal" and qb == QB - 1 and h == HL - 1
                            and nch > 2):
                        # very last head: put the cheap DB pair last so the
                        # final exp+PV before the norm that gates wo(QB-1)
                        # is as short as possible
                        chunks_h = ([chunks[0], chunks[2]] + chunks[3:]
                                    + [chunks[1]])
                    for ci, (kjs, kind) in enumerate(chunks_h):
                        s_ps = pmm.tile([P, 2, QBS], F32, tag="s",
                                        name=f"s_{qb}_{h}_{ci}")
                        es = esp.tile([P, 2, QBS], F16, tag="es",
                                      name=f"es_{qb}_{h}_{ci}")
                        if kind in ('DA', 'DB'):
                            # Both tiles of the pair compute scores over the
                            # pair's union range [lo0:QBS] (the second
                            # tile's leading 128 cols are live-but-unused:
                            # real finite values so the single union exp
                            # never reads undefined PSUM).  Only the PV is
                            # trimmed per-tile.
                            base = 0 if kind == 'DA' else 2
                            lo0 = P * base
                            for i, kj in enumerate(kjs):
                                nc.tensor.matmul(
                                    s_ps[:, i, lo0:QBS],
                                    kT_sb[po:po + DK, pair,
                                          kj * P:(kj + 1) * P],
                                    qT_sb[po:po + DK, pair,
                                          qb * QBS + lo0:(qb + 1) * QBS],
                                    start=True, stop=True)
                            nc.scalar.activation(es[:, 0:2, lo0:QBS],
                                                 s_ps[:, 0:2, lo0:QBS],
                                                 AF.Exp, scale=0.125)
                            for i in range(2):
                                lo = P * (base + i)
                                nc.vector.tensor_mul(
                                    es[:, i, lo:lo + P], es[:, i, lo:lo + P],
                                    mask4[:, base + i, lo:lo + P])
                        else:
                            for i, kj in enumerate(kjs):
                                nc.tensor.matmul(
                                    s_ps[:, i, :],
                                    kT_sb[po:po + DK, pair,
                                          kj * P:(kj + 1) * P],
                                    qT_h, start=True, stop=True)
                            n = len(kjs)
                            nc.scalar.activation(es[:, :n, :], s_ps[:, :n, :],
                                                 AF.Exp, scale=0.125)
                            if mode == "mask":
                                for i, kj in enumerate(kjs):
                                    m_t = esp.tile([P, QBS], F16, tag="mt")
                                    nc.sync.dma_start(
                                        m_t[:],
                                        maskt[kj * P:(kj + 1) * P, qsl])
                                    nc.vector.tensor_mul(es[:, i, :],
                                                         es[:, i, :],
                                                         m_t[:])
                        for _ in range(slot_pops.get(ci, 0)):
                            if fillers:
                                fillers.pop(0)()
                        if len(pending) >= cap:
                            pump()
                        def make_pv(o_ps, h, ci, kjs, kind, es):
                            def go():
                                if kind in ('DA', 'DB'):
                                    base = 0 if kind == 'DA' else 2
                                    for i, kj in enumerate(kjs):
                                        lo = P * (base + i)
                                        nc.tensor.matmul(
                                            o_ps[:, lo:QBS],
                                            v_sb[:, kj, h, :],
                                            es[:, i, lo:QBS],
                                            start=(ci == 0 and i == 0),
                                            stop=(ci == nch - 1
                                                  and i == len(kjs) - 1))
                                    return
                                for i, kj in enumerate(kjs):
                                    nc.tensor.matmul(
                                        o_ps[:], v_sb[:, kj, h, :],
                                        es[:, i, :],
                                        start=(ci == 0 and i == 0),
                                        stop=(ci == nch - 1
                                              and i == len(kjs) - 1))
                            return go
                        after = (make_norm(o_ps, pair, po)
                                 if ci == nch - 1 else None)
                        pending.append(
                            (make_pv(o_ps, h, ci, kjs, kind, es), after))
                while pending:
                    pump()
                for f in fillers:
                    f()

            # ---- output projection for one token chunk ----
            def wo_pieces(qb, tail=False):
                def tt_piece(tl):
                    def go():
                        _emit_wo_tt(qb, tl, tail=tail)
                    return go
                return [tt_piece(tl) for tl in range(KPB)]

            def emit_wo(qb, tail=False):
                for piece in wo_pieces(qb, tail=tail):
                    piece()

            def _emit_wo_tt(qb, tl, tail=False):
                    tt = qb * KPB + tl
                    last = tail and tl == KPB - 1
                    if last:
                        # ofc-outer so the first output chunk's copy + DMA
                        # overlap the second chunk's matmuls at the very
                        # end; each chunk accumulates in an o-tag PSUM bank
                        # (both free after the final norm) so the last tile
                        # never waits on the s-tag rotation behind the
                        # other wo tiles
                        for ofc in range(OFC):
                            osl = slice(ofc * OFS, (ofc + 1) * OFS)
                            wp = pop.tile([P, QBS], F32, tag="o",
                                          name=f"wlast_{ofc}")
                            for ft in range(FT):
                                nc.tensor.matmul(wp[:, 0:OFS],
                                                 oT_sb[:, ft, tt * P:(tt + 1) * P],
                                                 wo_sb[:, ft, osl],
                                                 start=(ft == 0),
                                                 stop=(ft == FT - 1))
                            o_out = osb.tile([P, OFC, OFS], F16, tag="oo",
                                             name=f"oo_{tt}_{ofc}")
                            if ofc % 2 == 0:
                                nc.scalar.activation(o_out[:, 0, :],
                                                     wp[:, 0:OFS],
                                                     AF.Identity, scale=1.0)
                                nc.scalar.dma_start(
                                    out[tt * P:(tt + 1) * P, osl],
                                    o_out[:, 0, :])
                            else:
                                nc.vector.tensor_copy(o_out[:, 0, :],
                                                      wp[:, 0:OFS])
                                nc.sync.dma_start(
                                    out[tt * P:(tt + 1) * P, osl],
                                    o_out[:, 0, :])
                        return
                    w_ps = pmm.tile([P, 2, QBS], F32, tag="s", name=f"w_{tt}")
                    # ft outer so each oT weight tile is loaded once for
                    # both output-feature chunks
                    for ft in range(FT):
                        for ofc in range(OFC):
                            osl = slice(ofc * OFS, (ofc + 1) * OFS)
                            nc.tensor.matmul(w_ps[:, ofc, 0:OFS],
                                             oT_sb[:, ft, tt * P:(tt + 1) * P],
                                             wo_sb[:, ft, osl],
                                             start=(ft == 0), stop=(ft == FT - 1))
                    o_out = osb.tile([P, OFC, OFS], F16, tag="oo",
                                     name=f"oo_{tt}")
                    if tail and tl % 2 == 0:
                        nc.scalar.activation(o_out[:],
                                             w_ps[:, 0:OFC, 0:OFS],
                                             AF.Identity, scale=1.0)
                    else:
                        nc.vector.tensor_copy(o_out[:], w_ps[:, 0:OFC, 0:OFS])
                    nc.sync.dma_start(
                        out[tt * P:(tt + 1) * P, :],
                        o_out[:].rearrange("p c o -> p (c o)"))

            if mode == "causal":
                v0_pieces = emit_chunk0(xk0, xq0, xv0)
                tiles1 = emit_proj_dma(1)
                p1 = proj_pieces(1, tiles1)
                rest = []
                for qb in range(QB):
                    fill_from, cap = 0, 5
                    if qb == 0:
                        tiles2 = emit_proj_dma(2)
                        emit_wo_dma()
                        p2 = proj_pieces(2, tiles2)
                        # qb-0 runs right after the K/Q projections; the
                        # chunk-0 V tiles and all of proj(1) interleave as
                        # fillers while the xv/chunk-1 DMAs land.  cap=4
                        # (two heads of PV lag) keeps the PE busy across
                        # the DMA-paced stretch; fillers start at head 1
                        # so the v tiles precede the first PV pops.
                        fillers, rest = v0_pieces + p1 + p2[:2], p2[2:]
                        fill_from, cap = 1, 5
                    elif qb == 1:
                        tiles3 = emit_proj_dma(3)
                        p3 = proj_pieces(3, tiles3)
                        fillers = rest + p3[:2]
                        rest = p3[2:]
                    elif qb == 2:
                        fillers, rest = rest + wo_pieces(0), []
                    else:
                        fillers = wo_pieces(1) + wo_pieces(2)
                    # last block: keep the final head filler-free so its
                    # scores (which pace the serial ACT exp stream that
                    # gates the tail) issue as early as possible
                    fill_to = HL - 1 if qb == QB - 1 else HL
                    emit_attn(qb, fillers, fill_from=fill_from, cap=cap,
                              fill_to=fill_to)
                emit_wo(QB - 1, tail=True)
            else:
                for ch in range(QB):
                    emit_proj(ch, (xk0, xq0, xv0) if ch == 0 else None)
                for qb in range(QB):
                    emit_attn(qb)
                    emit_wo(qb, tail=(qb == QB - 1))

            opool_cm.__exit__(None, None, None)
            pool_cm.__exit__(None, None, None)

    nc.compile()
    return nc


_PROGRAMS = {}


def _get_program(mode, seq=SEQ, d_model=D_MODEL, num_heads=NUM_HEADS):
    key = (mode, seq, d_model, num_heads)
    if key not in _PROGRAMS:
        _PROGRAMS[key] = build_program(seq, d_model, num_heads, mode)
    return _PROGRAMS[key]


def _detect_mode(mask, seq):
    m = np.asarray(mask)
    if (m != 0).all():
        return "dense"
    tril = np.tril(np.ones((seq, seq), np.int8))
    if np.array_equal((m != 0).astype(np.int8), tril):
        return "causal"
    return "mask"


def _x_layout(x, seq, d_model):
    """[seq, d_model] -> [P, QB*IN_T*QBS] partition-major fp16 so every
    chunk DMA is contiguous per partition."""
    IN_T = d_model // P
    QBS = min(512, seq)
    QB = seq // QBS
    a = np.ascontiguousarray(x.T).astype(np.float16)       # [d_model, seq]
    a = a.reshape(IN_T, P, QB, QBS).transpose(1, 2, 0, 3)  # [P, QB, IN_T, QBS]
    return np.ascontiguousarray(a.reshape(P, QB * IN_T * QBS))


def _w_layout(w_slice):
    """[FL, d_model] weight slice -> wT [d_model, FL] -> [P, IN_T*FL]."""
    FL, d_model = w_slice.shape
    IN_T = d_model // P
    a = np.ascontiguousarray(w_slice.T).astype(np.float16)  # [d_model, FL]
    a = a.reshape(IN_T, P, FL).transpose(1, 0, 2)
    return np.ascontiguousarray(a.reshape(P, IN_T * FL))


def _wo_layout(wo_slice):
    """[d_model, FL] Wo columns slice -> woT [FL, d_model] -> [P, FT*d_model]."""
    d_model, FL = wo_slice.shape
    FT = FL // P
    a = np.ascontiguousarray(wo_slice.T).astype(np.float16)  # [FL, d_model]
    a = a.reshape(FT, P, d_model).transpose(1, 0, 2)
    return np.ascontiguousarray(a.reshape(P, FT * d_model))


def prep_inputs(Q, K, V, mask, Wq, bq, Wk, bk, Wv, bv, Wo, bo,
                num_heads=NUM_HEADS, mode=None):
    batch, seq, d_model = Q.shape
    HL = num_heads // 2
    FL = HL * (d_model // num_heads)
    PAIRS = HL // 2
    if mode is None:
        mode = _detect_mode(mask, seq)
    maskt = None
    if mode == "mask":
        maskt = np.ascontiguousarray(
            (np.asarray(mask) != 0).astype(np.float16).T)
    in_maps = []
    for b in range(batch):
        xtq = _x_layout(Q[b], seq, d_model)
        xtk = _x_layout(K[b], seq, d_model)
        xtv = _x_layout(V[b], seq, d_model)
        for half in range(2):
            fsl = slice(half * FL, (half + 1) * FL)
            bkq = np.concatenate(
                [bk[fsl].reshape(PAIRS, P).T, bq[fsl].reshape(PAIRS, P).T],
                axis=1)
            im = {
                "xtq": xtq, "xtk": xtk, "xtv": xtv,
                "wqt": _w_layout(Wq[fsl, :]),
                "wkt": _w_layout(Wk[fsl, :]),
                "wvt": _w_layout(Wv[fsl, :]),
                "bkq": np.ascontiguousarray(bkq).astype(np.float32),
                "bvrow": bv[fsl].reshape(1, FL).astype(np.float16),
                "wot": _wo_layout(Wo[:, fsl]),
            }
            if maskt is not None:
                im["maskt"] = maskt
            in_maps.append(im)
    return in_maps, mode


def _install_trace_hooks():
    """Provide antenv.axon_hooks (missing in this image) so that
    run_bass_kernel_spmd(trace=True) can capture NTFF profiles via the
    axon PJRT .so.  Bench-only; the graded path never enables tracing."""
    import contextlib
    import ctypes
    import types
    try:
        from antenv import axon_hooks  # noqa: F401
        return
    except ImportError:
        pass
    lib = ctypes.CDLL("/opt/axon/libaxon_pjrt.so")
    if not hasattr(lib, "axon_start_nrt_profile"):
        return
    lib.axon_start_nrt_profile.argtypes = [ctypes.POINTER(ctypes.c_int64),
                                           ctypes.c_size_t]
    lib.axon_start_nrt_profile.restype = ctypes.c_int64
    lib.axon_stop_nrt_profile.argtypes = [ctypes.c_char_p]
    lib.axon_stop_nrt_profile.restype = ctypes.c_int64

    @contextlib.contextmanager
    def _hook(output_dir, device_ids):
        import jax
        jax.devices()
        if device_ids:
            ids = (ctypes.c_int64 * len(device_ids))(*device_ids)
            rc = lib.axon_start_nrt_profile(ids, len(device_ids))
        else:
            rc = lib.axon_start_nrt_profile(None, 0)
        if rc != 0:
            raise RuntimeError(f"axon_start_nrt_profile rc={rc}")
        try:
            yield
        finally:
            n = lib.axon_stop_nrt_profile(str(output_dir).encode())
            print(f"profile: {n} file(s) written to {output_dir}", file=sys.stderr)

    mod = types.ModuleType("antenv.axon_hooks")
    mod.get_axon_ntff_profile_hook = lambda: _hook
    mod.set_axon_ntff_profile_hook = lambda h: None
    sys.modules["antenv.axon_hooks"] = mod
    import concourse.bass_utils as bu
    bu.upload_artifacts = lambda tmpdir: f"local:{tmpdir}"


def kernel(Q, K, V, mask, Wq, bq, Wk, bk, Wv, bv, Wo, bo):
    global LAST_EXEC_NS, LAST_RESULTS
    Q = np.asarray(Q); K = np.asarray(K); V = np.asarray(V)
    mask = np.asarray(mask)
    Wq = np.asarray(Wq, np.float32); bq = np.asarray(bq, np.float32)
    Wk = np.asarray(Wk, np.float32); bk = np.asarray(bk, np.float32)
    Wv = np.asarray(Wv, np.float32); bv = np.asarray(bv, np.float32)
    Wo = np.asarray(Wo, np.float32); bo = np.asarray(bo, np.float32)
    batch, seq, d_model = Q.shape

    in_maps, mode = prep_inputs(Q, K, V, mask, Wq, bq, Wk, bk, Wv, bv, Wo, bo)
    nc = _get_program(mode, seq, d_model, NUM_HEADS)

    trace = bool(os.environ.get("KBENCH_TRACE"))
    tmpdir = os.environ.get("KBENCH_TRACE_DIR") or None
    if trace:
        _install_trace_hooks()
    res = run_bass_kernel_spmd(nc, in_maps, list(range(N_CORES)), trace=trace,
                               tmpdir=tmpdir)
    LAST_EXEC_NS = res.exec_time_ns
    LAST_RESULTS = res
    out = np.empty((batch, seq, d_model), np.float32)
    for b in range(batch):
        out[b] = (res.results[2 * b]["out"].astype(np.float32)
                  + res.results[2 * b + 1]["out"].astype(np.float32) + bo)
    return out


# revision 65
# speedup vs baseline: 1.0135x; 1.0135x over previous
"""Multi-head attention TRN2 kernel (8 NeuronCores).

Sharding: core (2b + h2) handles batch b (of 4) and head-half h2 (8 of 16
heads).  Each core projects its batch's Q/K/V through its 512-column slice
of Wq/Wk/Wv, runs causal flash-attention for its 8 heads, and computes a
partial output projection through its 512 rows of Wo^T.  The two partial
outputs per batch are summed on the host (the "all-reduce after W_o"),
plus the output bias.

All matmuls run in fp16 with fp32 PSUM accumulation.  Scores are computed
transposed (S^T[kj, qi] = kT.T @ qT) so the softmax sum comes for free from
a ones-column appended to V (padded to 128 columns so LDWEIGHTS gets fast
weight load), and the attention output lands f-major, which is exactly the
lhsT layout the Wo matmul needs.  Normalisation (divide by the softmax sum)
is a DVE fast-reciprocal (read straight from PSUM) + GPSIMD
partition-broadcast + DVE multiply.

Causal handling: per 512-token qi block the four diagonal 512x512 kj tiles
are trimmed to their live qi ranges (widths 512/384/256/128 for scores,
exp and PV; PSUM start zeroes the whole 2KB zero region so partial-width
accumulation is safe), and only the 128-wide boundary triangle of each
tile is masked.  Off-diagonal upper tiles are skipped entirely.

The schedule is a flat (head, chunk) software pipeline per qi block with
a deep pending-PV queue (lookahead 5, spanning head boundaries) so every
PV's es-wait resolves long before the PE reaches it; each head's
normalisation is emitted when its final PV pops.  Chunk-0 projections are
emitted K-first with host-side tensors laid out partition-major (every
DMA contiguous per partition, cheap descriptor generation) and startup
triggers split across the two HWDGE queues (SP + ACT).  Projection
pieces for chunks 1..3 and the Wo pieces for qi blocks 0..2 are spread
as PE filler across the attention blocks in proportion to each block's
PE-vs-ACT deficit (late blocks have more exp work per matmul); the k/q
projection biases are split across the ACT/DVE engines.  Partial outputs
stream back as fp16 and the two per-batch partials are summed on the
host.
"""

import os
import sys

sys.path.insert(0, "/opt/trn_rl_repo")

import numpy as np

import concourse.bass as bass
import concourse.mybir as mybir
import concourse.tile as tile
from concourse import bacc
from concourse.bass_utils import run_bass_kernel_spmd

F16 = mybir.dt.float16
F32 = mybir.dt.float32
P = 128

# Problem constants (full size).
D_MODEL = 1024
NUM_HEADS = 16
DK = D_MODEL // NUM_HEADS  # 64
BATCH = 4
SEQ = 2048
N_CORES = 8

LAST_EXEC_NS = None
LAST_RESULTS = None


def build_program(seq=SEQ, d_model=D_MODEL, num_heads=NUM_HEADS, mode="causal"):
    """Build the per-core Bass program.  Uniform across cores (SPMD).

    mode: "causal" (tril mask, block-skip + trimmed diagonal),
          "dense"  (no mask),
          "mask"   (arbitrary 0/1 mask, multiplicative, streamed from DRAM).
    """
    assert d_model % 256 == 0 and seq % P == 0
    HL = num_heads // 2              # local heads per core
    PAIRS = HL // 2                  # head-pairs (128 partitions each)
    FL = HL * DK                     # local features (columns of W slices)
    IN_T = d_model // P              # input-dim tiles
    FT = FL // P                     # local f tiles
    TT = seq // P                    # token tiles
    QBS = min(512, seq)              # qi block size
    QB = seq // QBS                  # qi blocks
    KJ = seq // P                    # key tiles
    KPB = QBS // P                   # key tiles per qi block (diag width)
    OFC = (d_model + 511) // 512     # output-feature chunks
    OFS = min(512, d_model)
    assert PAIRS >= 1 and FT >= 1 and QB >= 1

    nc = bacc.Bacc()
    # x tensors partition-major: [P, QB*IN_T*QBS], chunk ch contiguous.
    xtq = nc.declare_dram_parameter("xtq", [P, QB * IN_T * QBS], F16,
                                    isOutput=False)
    xtk = nc.declare_dram_parameter("xtk", [P, QB * IN_T * QBS], F16,
                                    isOutput=False)
    xtv = nc.declare_dram_parameter("xtv", [P, QB * IN_T * QBS], F16,
                                    isOutput=False)
    # weights partition-major: [P, IN_T*FL] / [P, FT*d_model].
    wqt = nc.declare_dram_parameter("wqt", [P, IN_T * FL], F16, isOutput=False)
    wkt = nc.declare_dram_parameter("wkt", [P, IN_T * FL], F16, isOutput=False)
    wvt = nc.declare_dram_parameter("wvt", [P, IN_T * FL], F16, isOutput=False)
    bkq = nc.declare_dram_parameter("bkq", [P, 2 * PAIRS], F32, isOutput=False)
    bvrow = nc.declare_dram_parameter("bvrow", [1, FL], F16, isOutput=False)
    wot = nc.declare_dram_parameter("wot", [P, FT * d_model], F16,
                                    isOutput=False)
    if mode == "mask":
        maskt = nc.declare_dram_parameter("maskt", [seq, seq], F16,
                                          isOutput=False)
    out = nc.declare_dram_parameter("out", [seq, d_model], F16, isOutput=True)

    AF = mybir.ActivationFunctionType

    def x_chunk_ap(xt, ch, it0=0, it1=None):
        it1 = IN_T if it1 is None else it1
        base = ch * IN_T * QBS
        return xt[:, base + it0 * QBS:base + it1 * QBS].rearrange(
            "p (it t) -> p it t", it=it1 - it0)

    with tile.TileContext(nc) as tc:
        with (
            tc.tile_pool(name="const", bufs=1) as cpool,
            tc.tile_pool(name="big", bufs=1) as big,
            tc.tile_pool(name="xs", bufs=2) as xs,
            tc.tile_pool(name="es", bufs=8) as esp,
            tc.tile_pool(name="ep", bufs=2) as epi,
            tc.tile_pool(name="osb", bufs=4) as osb,
        ):
            # ---- startup DMAs.  Both HWDGE queues (SP + ACT) carry the
            # K-path first (wk/xk split between them) so the first
            # accumulation groups are fed at full aggregate bandwidth,
            # then Q, then V.
            wk_sb = cpool.tile([P, IN_T, FL], F16, name="wk")
            xk0 = xs.tile([P, IN_T, QBS], F16, tag="xk", name="xk_0")
            H_IT = IN_T // 2
            nc.sync.dma_start(wk_sb[:, 0:H_IT, :],
                              wkt[:, 0:H_IT * FL].rearrange(
                                  "p (it f) -> p it f", it=H_IT))
            nc.scalar.dma_start(xk0[:, 0:H_IT, :], x_chunk_ap(xtk, 0, 0, H_IT))
            nc.sync.dma_start(wk_sb[:, H_IT:IN_T, :],
                              wkt[:, H_IT * FL:IN_T * FL].rearrange(
                                  "p (it f) -> p it f", it=IN_T - H_IT))
            nc.scalar.dma_start(xk0[:, H_IT:IN_T, :], x_chunk_ap(xtk, 0, H_IT))
            bkq_sb = cpool.tile([P, 2 * PAIRS], F32)
            nc.scalar.dma_start(bkq_sb[:], bkq[:, :])
            wq_sb = cpool.tile([P, IN_T, FL], F16, name="wq")
            nc.sync.dma_start(wq_sb[:],
                              wqt[:].rearrange("p (it f) -> p it f", it=IN_T))
            xq0 = xs.tile([P, IN_T, QBS], F16, tag="xq", name="xq_0")
            nc.scalar.dma_start(xq0[:], x_chunk_ap(xtq, 0))
            wv_sb = cpool.tile([P, IN_T, FL], F16, name="wv")
            nc.sync.dma_start(wv_sb[:],
                              wvt[:].rearrange("p (it f) -> p it f", it=IN_T))
            xv0 = xs.tile([P, IN_T, QBS], F16, tag="xv", name="xv_0")
            nc.scalar.dma_start(xv0[:], x_chunk_ap(xtv, 0))
            bv_sb = cpool.tile([1, FL], F16)
            nc.scalar.dma_start(bv_sb[:], bvrow[:, :])

            def bk_col(pair):
                return bkq_sb[:, pair:pair + 1]

            def bq_col(pair):
                return bkq_sb[:, PAIRS + pair:PAIRS + pair + 1]

            # wo is needed only ~150us in; its DMA is issued at the start
            # of the qb-0 attention block.
            wo_sb = cpool.tile([P, FT, d_model], F16)

            def emit_wo_dma():
                nc.sync.dma_start(
                    wo_sb[:],
                    wot[:].rearrange("p (ft o) -> p ft o", ft=FT))

            # ---- non-DMA constants ----
            ones1 = cpool.tile([1, P], F16)
            nc.gpsimd.memset(ones1[:], 1.0)
            # warm the ACT exp table early (one-time ~1.3us load)
            es_warm = esp.tile([1, 8], F16, tag="warm")
            nc.scalar.activation(es_warm[:], ones1[0:1, 0:8], AF.Exp, scale=1.0)
            bvb = cpool.tile([P, FL], F16)
            nc.gpsimd.partition_broadcast(bvb[:], bv_sb[0:1, :])
            # 4 diagonal 0/1 strips: strip j keeps (qi - kj_local - 128*j >= 0)
            mask4 = cpool.tile([P, KPB, QBS], F16)
            nc.gpsimd.memset(mask4[:], 1.0)
            for j in range(KPB):
                nc.gpsimd.affine_select(
                    out=mask4[:, j, :], in_=mask4[:, j, :],
                    compare_op=mybir.AluOpType.is_ge,
                    fill=0.0, base=-P * j,
                    pattern=[[1, QBS]], channel_multiplier=-1)

            # ---- persistent activations ----
            qT_sb = big.tile([P, PAIRS, seq], F16)   # [2-head f, pair, tok]
            kT_sb = big.tile([P, PAIRS, seq], F16)
            v_sb = big.tile([P, TT, HL, P], F16)  # [tok_in_tile, kj, h, d|1|pad]
            oT_sb = big.tile([P, FT, seq], F16)      # attention out, f-major

            nc.gpsimd.memset(v_sb[:], 0.0)
            nc.gpsimd.memset(v_sb[:, :, :, DK:DK + 1], 1.0)

            # One shared PSUM pool: tag "s" [128,2,512]x3 = 6 banks (scores,
            # projections, Wo) + tag "o" [128,512]x2 = 2 banks.
            pool_cm = tc.tile_pool(name="pmm", bufs=3, space="PSUM")
            pmm = pool_cm.__enter__()
            opool_cm = tc.tile_pool(name="po", bufs=2, space="PSUM")
            pop = opool_cm.__enter__()

            def emit_proj_dma(ch):
                    # xq/xv ride the ACT HWDGE queue: the SP queue
                    # otherwise carries ~3x the bytes and chunk V data
                    # lands several us after the PE needs it
                    xk_t = xs.tile([P, IN_T, QBS], F16, tag="xk",
                                   name=f"xk_{ch}")
                    nc.sync.dma_start(xk_t[:], x_chunk_ap(xtk, ch))
                    xq_t = xs.tile([P, IN_T, QBS], F16, tag="xq",
                                   name=f"xq_{ch}")
                    nc.scalar.dma_start(xq_t[:], x_chunk_ap(xtq, ch))
                    xv_t = xs.tile([P, IN_T, QBS], F16, tag="xv",
                                   name=f"xv_{ch}")
                    nc.scalar.dma_start(xv_t[:], x_chunk_ap(xtv, ch))
                    return xk_t, xq_t, xv_t

            def proj_pieces(ch, tiles=None):
                    tsl = slice(ch * QBS, (ch + 1) * QBS)
                    xk_t, xq_t, xv_t = tiles if tiles else emit_proj_dma(ch)
                    pieces = []
                    def qk_piece(pair):
                        def go():
                            _emit_qk_pair(ch, tsl, xk_t, xq_t, pair)
                        return go
                    def v_piece(tl):
                        def go():
                            _emit_v_tile(ch, tsl, xv_t, tl)
                        return go
                    for pair in range(PAIRS):
                        pieces.append(qk_piece(pair))
                    for tl in range(KPB):
                        pieces.append(v_piece(tl))
                    return pieces

            def emit_proj(ch, tiles=None):
                    for piece in proj_pieces(ch, tiles):
                        piece()

            def _emit_qk_pair(ch, tsl, xk_t, xq_t, pair):
                        fsl = slice(pair * P, (pair + 1) * P)
                        qk_ps = pmm.tile([P, 2, QBS], F32, tag="s",
                                         name=f"qk_{ch}_{pair}")
                        for it in range(IN_T):
                            nc.tensor.matmul(qk_ps[:, 0, :],
                                             wk_sb[:, it, fsl],
                                             xk_t[:, it, :],
                                             start=(it == 0), stop=(it == IN_T - 1))
                        for it in range(IN_T):
                            nc.tensor.matmul(qk_ps[:, 1, :],
                                             wq_sb[:, it, fsl],
                                             xq_t[:, it, :],
                                             start=(it == 0), stop=(it == IN_T - 1))
                        # bias + f32->f16 cast, split across ACT and DVE so
                        # neither the exp stream nor the norm chains (which
                        # gate o-PSUM rotation) queue behind both
                        nc.scalar.activation(kT_sb[:, pair, tsl],
                                             qk_ps[:, 0, :], AF.Identity,
                                             bias=bk_col(pair), scale=1.0)
                        nc.vector.tensor_scalar_add(qT_sb[:, pair, tsl],
                                                    qk_ps[:, 1, :],
                                                    bq_col(pair))

            def _emit_v_tile(ch, tsl, xv_t, tl):
                        v_ps = pmm.tile([P, 2, QBS], F32, tag="s",
                                        name=f"v_{ch}_{tl}")
                        for it in range(IN_T):
                            nc.tensor.matmul(
                                v_ps[:, 0, 0:FL],
                                xv_t[:, it, tl * P:(tl + 1) * P],
                                wv_sb[:, it, :],
                                start=(it == 0), stop=(it == IN_T - 1))
                        tt = ch * KPB + tl
                        nc.vector.tensor_tensor(
                            v_sb[:, tt, :, 0:DK],
                            v_ps[:, 0, 0:FL].rearrange("p (h d) -> p h d", h=HL),
                            bvb[:].rearrange("p (h d) -> p h d", h=HL),
                            mybir.AluOpType.add)

            def emit_chunk0(xk_t, xq_t, xv_t):
                    """Chunk-0 projections: K for all pairs, then Q pairs
                    0-1, so the first matmuls only wait on the xk/wk then
                    xq/wq DMAs.  Q pairs 2-3 and all V tiles are returned
                    as pieces scheduled inside the qb-0 attention stream
                    (the xv DMA is the last to land, and attention heads
                    0-3 only need q/k pairs 0-1)."""
                    tsl = slice(0, QBS)

                    def qk0_grp(wn, grp):
                        w_sb, xt, b_col, t_sb = (
                            (wk_sb, xk_t, bk_col, kT_sb),
                            (wq_sb, xq_t, bq_col, qT_sb))[wn]
                        ps = pmm.tile([P, 2, QBS], F32, tag="s",
                                      name=f"p0_{wn}_{grp}")
                        for i in range(2):
                            pair = grp * 2 + i
                            fsl = slice(pair * P, (pair + 1) * P)
                            for it in range(IN_T):
                                nc.tensor.matmul(
                                    ps[:, i, :],
                                    w_sb[:, it, fsl],
                                    xt[:, it, :],
                                    start=(it == 0), stop=(it == IN_T - 1))
                        for i in range(2):
                            pair = grp * 2 + i
                            if wn == 0:
                                nc.scalar.activation(
                                    t_sb[:, pair, tsl], ps[:, i, :],
                                    AF.Identity, bias=b_col(pair),
                                    scale=1.0)
                            else:
                                nc.vector.tensor_scalar_add(
                                    t_sb[:, pair, tsl], ps[:, i, :],
                                    b_col(pair))

                    for grp in range(PAIRS // 2):
                        qk0_grp(0, grp)
                    for grp in range(PAIRS // 2):
                        qk0_grp(1, grp)

                    def v_piece(tl):
                        def go():
                            _emit_v_tile(0, tsl, xv_t, tl)
                        return go
                    return [v_piece(tl) for tl in range(KPB)]

            # ---- attention for qi block qb, all local heads ----
            def attn_chunks(qb):
                """Chunk list [(kjs, kind)] for one qi block.  kind:
                'O' off-diagonal (full), 'DA' leading diag pair (widths
                512/384), 'DB' trailing diag pair (widths 256/128).  Diag
                tiles are trimmed to their live qi range; PSUM start=True
                zeroes the whole 2KB zero region so partial-width
                accumulation is safe, and stop bookkeeping is region-
                granular so a partial-width stop closes the group."""
                if mode != "causal":
                    chunks = []
                    for kj0 in range(0, KJ, 2):
                        kjs = (kj0, kj0 + 1) if kj0 + 1 < KJ else (kj0,)
                        chunks.append((kjs, 'O'))
                    return chunks
                d0 = qb * KPB
                DA = ((d0, d0 + 1), 'DA')
                DB = ((d0 + 2, d0 + 3), 'DB')
                if qb == 0:
                    return [DA, DB]
                # Diagonal (masked) chunks go second/third so their mask
                # multiplies never sit in the exposed exp->PV tail of the
                # head; the last off-diagonal chunk carries the stop.
                offs = [((kj0, kj0 + 1), 'O') for kj0 in range(0, d0, 2)]
                return [offs[0], DB, DA] + offs[1:]

            def emit_attn(qb, fillers=(), fill_from=0, cap=2):
                fillers = list(fillers)
                qsl = slice(qb * QBS, (qb + 1) * QBS)
                chunks = attn_chunks(qb)
                nch = len(chunks)
                # odd chunk indices are filler slots (after that chunk's
                # scores, inside the exp->PV latency window)
                slots = [ci for ci in range(1, nch, 2)]
                # One flat (head, chunk) pipeline: the pending-PV queue
                # (lookahead `cap`) spans head boundaries, so the last PVs
                # of head h pop during head h+1's first scores instead of
                # stalling on their just-issued exps; each head's norm is
                # emitted when its final PV pops.
                pending = []  # (pv_fn, after_fn | None)

                def pump():
                    pv, after = pending.pop(0)
                    pv()
                    if after is not None:
                        after()

                def make_norm(o_ps, pair, po):
                    def go():
                        # normalise by the softmax sum (row DK), f-major:
                        # denominator row to a partition-0 tile for the
                        # custom-DVE reciprocal, GPSIMD-broadcast, then
                        # multiply straight from PSUM.
                        srow = epi.tile([1, QBS], F32, tag="srow")
                        nc.vector.tensor_copy(srow[:], o_ps[DK:DK + 1, :])
                        recip_row = epi.tile([1, QBS], F32, tag="recip_row")
                        nc.vector.reciprocal_approx_fast(recip_row[:],
                                                         srow[:])
                        recipb = epi.tile([DK, QBS], F32, tag="recipb")
                        nc.gpsimd.partition_broadcast(recipb[:],
                                                      recip_row[0:1, :])
                        nc.vector.tensor_mul(oT_sb[po:po + DK, pair, qsl],
                                             o_ps[0:DK, :], recipb[:])
                    return go

                for h in range(HL):
                    pair = h // 2
                    po = (h % 2) * DK
                    qT_h = qT_sb[po:po + DK, pair, qsl]
                    o_ps = pop.tile([P, QBS], F32, tag="o",
                                    name=f"o_{qb}_{h}")
                    # this head's share of the remaining fillers, spread
                    # over its slots (multiple pops per slot if needed)
                    if fillers and h >= fill_from:
                        nf = -(-len(fillers) // (HL - h))
                    else:
                        nf = 0
                    slot_pops = {}
                    for j in range(nf):
                        ci_s = slots[j % len(slots)]
                        slot_pops[ci_s] = slot_pops.get(ci_s, 0) + 1
                    chunks_h = chunks
                    if (mode == "causal" and qb == QB - 1 and h == HL - 1
                            and nch > 2):
                        # very last head: put the cheap DB pair last so the
                        # final exp+PV before the norm that gates wo(QB-1)
                        # is as short as possible
                        chunks_h = ([chunks[0], chunks[2]] + chunks[3:]
                                    + [chunks[1]])
                    for ci, (kjs, kind) in enumerate(chunks_h):
                        s_ps = pmm.tile([P, 2, QBS], F32, tag="s",
                                        name=f"s_{qb}_{h}_{ci}")
                        es = esp.tile([P, 2, QBS], F16, tag="es",
                                      name=f"es_{qb}_{h}_{ci}")
                        if kind in ('DA', 'DB'):
                            # Both tiles of the pair compute scores over the
                            # pair's union range [lo0:QBS] (the second
                            # tile's leading 128 cols are live-but-unused:
                            # real finite values so the single union exp
                            # never reads undefined PSUM).  Only the PV is
                            # trimmed per-tile.
                            base = 0 if kind == 'DA' else 2
                            lo0 = P * base
                            for i, kj in enumerate(kjs):
                                nc.tensor.matmul(
                                    s_ps[:, i, lo0:QBS],
                                    kT_sb[po:po + DK, pair,
                                          kj * P:(kj + 1) * P],
                                    qT_sb[po:po + DK, pair,
                                          qb * QBS + lo0:(qb + 1) * QBS],
                                    start=True, stop=True)
                            nc.scalar.activation(es[:, 0:2, lo0:QBS],
                                                 s_ps[:, 0:2, lo0:QBS],
                                                 AF.Exp, scale=0.125)
                            for i in range(2):
                                lo = P * (base + i)
                                nc.vector.tensor_mul(
                                    es[:, i, lo:lo + P], es[:, i, lo:lo + P],
                                    mask4[:, base + i, lo:lo + P])
                        else:
                            for i, kj in enumerate(kjs):
                                nc.tensor.matmul(
                                    s_ps[:, i, :],
                                    kT_sb[po:po + DK, pair,
                                          kj * P:(kj + 1) * P],
                                    qT_h, start=True, stop=True)
                            n = len(kjs)
                            nc.scalar.activation(es[:, :n, :], s_ps[:, :n, :],
                                                 AF.Exp, scale=0.125)
                            if mode == "mask":
                                for i, kj in enumerate(kjs):
                                    m_t = esp.tile([P, QBS], F16, tag="mt")
                                    nc.sync.dma_start(
                                        m_t[:],
                                        maskt[kj * P:(kj + 1) * P, qsl])
                                    nc.vector.tensor_mul(es[:, i, :],
                                                         es[:, i, :],
                                                         m_t[:])
                        for _ in range(slot_pops.get(ci, 0)):
                            if fillers:
                                fillers.pop(0)()
                        if len(pending) >= cap:
                            pump()
                        def make_pv(o_ps, h, ci, kjs, kind, es):
                            def go():
                                if kind in ('DA', 'DB'):
                                    base = 0 if kind == 'DA' else 2
                                    for i, kj in enumerate(kjs):
                                        lo = P * (base + i)
                                        nc.tensor.matmul(
                                            o_ps[:, lo:QBS],
                                            v_sb[:, kj, h, :],
                                            es[:, i, lo:QBS],
                                            start=(ci == 0 and i == 0),
                                            stop=(ci == nch - 1
                                                  and i == len(kjs) - 1))
                                    return
                                for i, kj in enumerate(kjs):
                                    nc.tensor.matmul(
                                        o_ps[:], v_sb[:, kj, h, :],
                                        es[:, i, :],
                                        start=(ci == 0 and i == 0),
                                        stop=(ci == nch - 1
                                              and i == len(kjs) - 1))
                            return go
                        after = (make_norm(o_ps, pair, po)
                                 if ci == nch - 1 else None)
                        pending.append(
                            (make_pv(o_ps, h, ci, kjs, kind, es), after))
                while pending:
                    pump()
                for f in fillers:
                    f()

            # ---- output projection for one token chunk ----
            def wo_pieces(qb, tail=False):
                def tt_piece(tl):
                    def go():
                        _emit_wo_tt(qb, tl, tail=tail)
                    return go
                return [tt_piece(tl) for tl in range(KPB)]

            def emit_wo(qb, tail=False):
                for piece in wo_pieces(qb, tail=tail):
                    piece()

            def _emit_wo_tt(qb, tl, tail=False):
                    tt = qb * KPB + tl
                    last = tail and tl == KPB - 1
                    if last:
                        # ofc-outer so the first output chunk's copy + DMA
                        # overlap the second chunk's matmuls at the very
                        # end; each chunk accumulates in an o-tag PSUM bank
                        # (both free after the final norm) so the last tile
                        # never waits on the s-tag rotation behind the
                        # other wo tiles
                        for ofc in range(OFC):
                            osl = slice(ofc * OFS, (ofc + 1) * OFS)
                            wp = pop.tile([P, QBS], F32, tag="o",
                                          name=f"wlast_{ofc}")
                            for ft in range(FT):
                                nc.tensor.matmul(wp[:, 0:OFS],
                                                 oT_sb[:, ft, tt * P:(tt + 1) * P],
                                                 wo_sb[:, ft, osl],
                                                 start=(ft == 0),
                                                 stop=(ft == FT - 1))
                            o_out = osb.tile([P, OFC, OFS], F16, tag="oo",
                                             name=f"oo_{tt}_{ofc}")
                            if ofc % 2 == 0:
                                nc.scalar.activation(o_out[:, 0, :],
                                                     wp[:, 0:OFS],
                                                     AF.Identity, scale=1.0)
                                nc.scalar.dma_start(
                                    out[tt * P:(tt + 1) * P, osl],
                                    o_out[:, 0, :])
                            else:
                                nc.vector.tensor_copy(o_out[:, 0, :],
                                                      wp[:, 0:OFS])
                                nc.sync.dma_start(
                                    out[tt * P:(tt + 1) * P, osl],
                                    o_out[:, 0, :])
                        return
                    w_ps = pmm.tile([P, 2, QBS], F32, tag="s", name=f"w_{tt}")
                    # ft outer so each oT weight tile is loaded once for
                    # both output-feature chunks
                    for ft in range(FT):
                        for ofc in range(OFC):
                            osl = slice(ofc * OFS, (ofc + 1) * OFS)
                            nc.tensor.matmul(w_ps[:, ofc, 0:OFS],
                                             oT_sb[:, ft, tt * P:(tt + 1) * P],
                                             wo_sb[:, ft, osl],
                                             start=(ft == 0), stop=(ft == FT - 1))
                    o_out = osb.tile([P, OFC, OFS], F16, tag="oo",
                                     name=f"oo_{tt}")
                    if tail and tl % 2 == 0:
                        nc.scalar.activation(o_out[:],
                                             w_ps[:, 0:OFC, 0:OFS],
                                             AF.Identity, scale=1.0)
                    else:
                        nc.vector.tensor_copy(o_out[:], w_ps[:, 0:OFC, 0:OFS])
                    nc.sync.dma_start(
                        out[tt * P:(tt + 1) * P, :],
                        o_out[:].rearrange("p c o -> p (c o)"))

            if mode == "causal":
                v0_pieces = emit_chunk0(xk0, xq0, xv0)
                tiles1 = emit_proj_dma(1)
                p1 = proj_pieces(1, tiles1)
                rest = []
                for qb in range(QB):
                    fill_from, cap = 0, 5
                    if qb == 0:
                        tiles2 = emit_proj_dma(2)
                        emit_wo_dma()
                        p2 = proj_pieces(2, tiles2)
                        # qb-0 runs right after the K/Q projections; the
                        # chunk-0 V tiles and all of proj(1) interleave as
                        # fillers while the xv/chunk-1 DMAs land.  cap=4
                        # (two heads of PV lag) keeps the PE busy across
                        # the DMA-paced stretch; fillers start at head 1
                        # so the v tiles precede the first PV pops.
                        fillers, rest = v0_pieces + p1 + p2[:2], p2[2:]
                        fill_from, cap = 1, 5
                    elif qb == 1:
                        tiles3 = emit_proj_dma(3)
                        p3 = proj_pieces(3, tiles3)
                        fillers = rest + p3[:2]
                        rest = p3[2:]
                    elif qb == 2:
                        fillers, rest = rest + wo_pieces(0), []
                    else:
                        fillers = wo_pieces(1) + wo_pieces(2)
                    emit_attn(qb, fillers, fill_from=fill_from, cap=cap)
                emit_wo(QB - 1, tail=True)
            else:
                for ch in range(QB):
                    emit_proj(ch, (xk0, xq0, xv0) if ch == 0 else None)
                for qb in range(QB):
                    emit_attn(qb)
                    emit_wo(qb, tail=(qb == QB - 1))

            opool_cm.__exit__(None, None, None)
            pool_cm.__exit__(None, None, None)

    nc.compile()
    return nc


_PROGRAMS = {}


def _get_program(mode, seq=SEQ, d_model=D_MODEL, num_heads=NUM_HEADS):
    key = (mode, seq, d_model, num_heads)
    if key not in _PROGRAMS:
        _PROGRAMS[key] = build_program(seq, d_model, num_heads, mode)
    return _PROGRAMS[key]


def _detect_mode(mask, seq):
    m = np.asarray(mask)
    if (m != 0).all():
        return "dense"
    tril = np.tril(np.ones((seq, seq), np.int8))
    if np.array_equal((m != 0).astype(np.int8), tril):
        return "causal"
    return "mask"


def _x_layout(x, seq, d_model):
    """[seq, d_model] -> [P, QB*IN_T*QBS] partition-major fp16 so every
    chunk DMA is contiguous per partition."""
    IN_T = d_model // P
    QBS = min(512, seq)
    QB = seq // QBS
    a = np.ascontiguousarray(x.T).astype(np.float16)       # [d_model, seq]
    a = a.reshape(IN_T, P, QB, QBS).transpose(1, 2, 0, 3)  # [P, QB, IN_T, QBS]
    return np.ascontiguousarray(a.reshape(P, QB * IN_T * QBS))


def _w_layout(w_slice):
    """[FL, d_model] weight slice -> wT [d_model, FL] -> [P, IN_T*FL]."""
    FL, d_model = w_slice.shape
    IN_T = d_model // P
    a = np.ascontiguousarray(w_slice.T).astype(np.float16)  # [d_model, FL]
    a = a.reshape(IN_T, P, FL).transpose(1, 0, 2)
    return np.ascontiguousarray(a.reshape(P, IN_T * FL))


def _wo_layout(wo_slice):
    """[d_model, FL] Wo columns slice -> woT [FL, d_model] -> [P, FT*d_model]."""
    d_model, FL = wo_slice.shape
    FT = FL // P
    a = np.ascontiguousarray(wo_slice.T).astype(np.float16)  # [FL, d_model]
    a = a.reshape(FT, P, d_model).transpose(1, 0, 2)
    return np.ascontiguousarray(a.reshape(P, FT * d_model))


def prep_inputs(Q, K, V, mask, Wq, bq, Wk, bk, Wv, bv, Wo, bo,
                num_heads=NUM_HEADS, mode=None):
    batch, seq, d_model = Q.shape
    HL = num_heads // 2
    FL = HL * (d_model // num_heads)
    PAIRS = HL // 2
    if mode is None:
        mode = _detect_mode(mask, seq)
    maskt = None
    if mode == "mask":
        maskt = np.ascontiguousarray(
            (np.asarray(mask) != 0).astype(np.float16).T)
    in_maps = []
    for b in range(batch):
        xtq = _x_layout(Q[b], seq, d_model)
        xtk = _x_layout(K[b], seq, d_model)
        xtv = _x_layout(V[b], seq, d_model)
        for half in range(2):
            fsl = slice(half * FL, (half + 1) * FL)
            bkq = np.concatenate(
                [bk[fsl].reshape(PAIRS, P).T, bq[fsl].reshape(PAIRS, P).T],
                axis=1)
            im = {
                "xtq": xtq, "xtk": xtk, "xtv": xtv,
                "wqt": _w_layout(Wq[fsl, :]),
                "wkt": _w_layout(Wk[fsl, :]),
                "wvt": _w_layout(Wv[fsl, :]),
                "bkq": np.ascontiguousarray(bkq).astype(np.float32),
                "bvrow": bv[fsl].reshape(1, FL).astype(np.float16),
                "wot": _wo_layout(Wo[:, fsl]),
            }
            if maskt is not None:
                im["maskt"] = maskt
            in_maps.append(im)
    return in_maps, mode


def _install_trace_hooks():
    """Provide antenv.axon_hooks (missing in this image) so that
    run_bass_kernel_spmd(trace=True) can capture NTFF profiles via the
    axon PJRT .so.  Bench-only; the graded path never enables tracing."""
    import contextlib
    import ctypes
    import types
    try:
        from antenv import axon_hooks  # noqa: F401
        return
    except ImportError:
        pass
    lib = ctypes.CDLL("/opt/axon/libaxon_pjrt.so")
    if not hasattr(lib, "axon_start_nrt_profile"):
        return
    lib.axon_start_nrt_profile.argtypes = [ctypes.POINTER(ctypes.c_int64),
                                           ctypes.c_size_t]
    lib.axon_start_nrt_profile.restype = ctypes.c_int64
    lib.axon_stop_nrt_profile.argtypes = [ctypes.c_char_p]
    lib.axon_stop_nrt_profile.restype = ctypes.c_int64

    @contextlib.contextmanager
    def _hook(output_dir, device_ids):
        import jax
        jax.devices()
        if device_ids:
            ids = (ctypes.c_int64 * len(device_ids))(*device_ids)
            rc = lib.axon_start_nrt_profile(ids, len(device_ids))
        else:
            rc = lib.axon_start_nrt_profile(None, 0)
        if rc != 0:
            raise RuntimeError(f"axon_start_nrt_profile rc={rc}")
        try:
            yield
        finally:
            n = lib.axon_stop_nrt_profile(str(output_dir).encode())
            print(f"profile: {n} file(s) written to {output_dir}", file=sys.stderr)

    mod = types.ModuleType("antenv.axon_hooks")
    mod.get_axon_ntff_profile_hook = lambda: _hook
    mod.set_axon_ntff_profile_hook = lambda h: None
    sys.modules["antenv.axon_hooks"] = mod
    import concourse.bass_utils as bu
    bu.upload_artifacts = lambda tmpdir: f"local:{tmpdir}"


def kernel(Q, K, V, mask, Wq, bq, Wk, bk, Wv, bv, Wo, bo):
    global LAST_EXEC_NS, LAST_RESULTS
    Q = np.asarray(Q); K = np.asarray(K); V = np.asarray(V)
    mask = np.asarray(mask)
    Wq = np.asarray(Wq, np.float32); bq = np.asarray(bq, np.float32)
    Wk = np.asarray(Wk, np.float32); bk = np.asarray(bk, np.float32)
    Wv = np.asarray(Wv, np.float32); bv = np.asarray(bv, np.float32)
    Wo = np.asarray(Wo, np.float32); bo = np.asarray(bo, np.float32)
    batch, seq, d_model = Q.shape

    in_maps, mode = prep_inputs(Q, K, V, mask, Wq, bq, Wk, bk, Wv, bv, Wo, bo)
    nc = _get_program(mode, seq, d_model, NUM_HEADS)

    trace = bool(os.environ.get("KBENCH_TRACE"))
    tmpdir = os.environ.get("KBENCH_TRACE_DIR") or None
    if trace:
        _install_trace_hooks()
    res = run_bass_kernel_spmd(nc, in_maps, list(range(N_CORES)), trace=trace,
                               tmpdir=tmpdir)
    LAST_EXEC_NS = res.exec_time_ns
    LAST_RESULTS = res
    out = np.empty((batch, seq, d_model), np.float32)
    for b in range(batch):
        out[b] = (res.results[2 * b]["out"].astype(np.float32)
                  + res.results[2 * b + 1]["out"].astype(np.float32) + bo)
    return out
